# revision 42
# baseline (speedup 1.0000x reference)
# Trainium2 Bass kernel for nn_Attention3 (unnormalized linear attention).
#
# Math: e_i = x @ W_i.T + b_i (i=1,2,3);  out = sigmoid((e1 @ e2.T @ e3) @ WO.T + bO)
# Since there is no softmax, (e1 @ e2.T) @ e3 == e1 @ (e2.T @ e3) where
# KV = e2.T @ e3 is only [64, 64].
#
# Sharding: the flattened [B*S, 512] = [16384, 512] rows are split into 8
# chunks of 2048 rows (cores 0-3 <- batch 0, cores 4-7 <- batch 1).  Every
# core streams its WHOLE batch to build the full KV^T = e3.T @ e2 locally --
# redundant compute, but fully deterministic: no collectives or cross-core
# synchronization (measured ncfw AllGather latency on this setup is 25-100us
# with heavy per-core skew, far worse than the extra DMA).  Each core's OWN
# 2048 rows are ordered first in its input so e1 and the output stage run on
# chunks 0-3 with uniform (SPMD) code.
#
# Precision: x is cast to float16 on the HOST, halving HBM read traffic
# (16.8 -> 8.4 MB/core) and removing the on-chip f32->f16 DVE casts that
# were ~23us of Vector time.  All matmuls run f16 operands / fp32 PSUM.
# The output is written f16 (sigmoid range (0,1): ~5e-4 rounding) and
# upcast on the host.  Biases ride in the f16 weight blob.
#
# Layouts: x arrives host-transposed and pre-tiled as [128, chunk, 2048] f16
# so every DMA reads contiguous 4 KiB per partition (16 KiB per partition
# per 4-chunk group DMA); the output is written back partition-major
# ([128, rows*4] f16) and un-permuted on the host.  All weights/biases
# arrive packed in one [128, 1922] f16 blob (single DMA) and are used as
# views -- no unpack copies.

import numpy as np

import concourse.bass as bass
import concourse.mybir as mybir
import concourse.tile as tile
from concourse import bacc
from concourse.bass_utils import run_bass_kernel_spmd

BATCH = 2
SEQ = 8192
DIN = 512
DE = 64
N_CORES = 8
ROWS = (BATCH * SEQ) // N_CORES  # 2048 output rows per core

# const blob layout (free-dim offsets, f16, [128, NB])
_OFF_W1T = 0          # [128, 4, 64]   w1t rearranged (kt p) d -> p kt d
_OFF_W23T = 256       # [128, 4, 128]  w23t rearranged
_OFF_IDENT = 768      # [128, 128]     identity
_OFF_B23 = 896        # [128, 1]       b2|b3 (per-partition)
_OFF_WOT = 897        # [64, 512]      WO.T (rows 0..63)
_OFF_B1 = 1409        # [64, 1]        b1 (rows 0..63)
_OFF_BO = 1410        # [1, 512]       bO (row 0)
_NB = 1922

TRACE = False
TRACE_KWARGS = {}
LAST_RESULT = None

_NC_CACHE = {}


def build_nc(rows=ROWS, n_cores=N_CORES):
    f32 = mybir.dt.float32
    f16 = mybir.dt.float16

    group = n_cores // 2  # cores per batch
    assert rows % 512 == 0
    own_chunks = rows // 512
    n_chunks = own_chunks * group  # whole batch streamed per core

    nc = bacc.Bacc(
        None,
        target_bir_lowering=False,
        debug=False,
        num_devices=n_cores,
    )

    xt = nc.dram_tensor("xt", [128, n_chunks * 2048], f16, kind="ExternalInput")
    wconst = nc.dram_tensor("wconst", [128, _NB], f16, kind="ExternalInput")
    # b2|b3 (col 0, 128 rows) and b1 (col 1, rows 0..63) — DVE tensor_scalar
    # requires a float32 scalar operand, so these ride outside the f16 blob.
    bias32 = nc.dram_tensor("bias32", [128, 2], f32, kind="ExternalInput")
    out = nc.dram_tensor("out", [128, rows * 4], f16, kind="ExternalOutput")

    xt_t = xt.ap().rearrange("p (j f) -> p j f", f=2048)  # [128, n_chunks, 2048]
    out_t = out.ap().rearrange("p (j f) -> p j f", f=2048)

    with tile.TileContext(nc) as tc:
        with (
            tc.tile_pool(name="consts", bufs=1) as consts,
            tc.tile_pool(name="persist", bufs=1) as persist,
            tc.tile_pool(name="kvps", bufs=1, space="PSUM") as kvps,
            tc.tile_pool(name="kvbps", bufs=1, space="PSUM") as kvbps,
            tc.tile_pool(name="small", bufs=1) as small,
        ):
            blob = consts.tile([128, _NB], f16)
            nc.sync.dma_start(out=blob, in_=wconst.ap())
            biases = consts.tile([128, 2], f32)
            nc.sync.dma_start(out=biases, in_=bias32.ap())

            sb_w1t = blob[:, _OFF_W1T : _OFF_W1T + 256].rearrange(
                "p (kt d) -> p kt d", kt=4
            )
            sb_w23t = blob[:, _OFF_W23T : _OFF_W23T + 512].rearrange(
                "p (kt d) -> p kt d", kt=4
            )
            sb_wot = blob[:DE, _OFF_WOT : _OFF_WOT + DIN]
            identity = blob[:, _OFF_IDENT : _OFF_IDENT + 128]
            sb_b23 = biases[:, 0:1]
            sb_b1 = biases[:DE, 1:2]
            sb_bo = blob[:1, _OFF_BO : _OFF_BO + DIN]

            # e1^T for the own rows, with a row of ones at partition DE so the
            # final matmul folds in the output bias (lhsT K = DE+1).
            e1t = persist.tile([128, rows], f16)
            nc.vector.memset(e1t[DE : DE + 1, :], 1.0)
            # M = KV @ WO.T in rows 0..63, bO in row DE.
            mmat = persist.tile([128, DIN], f16)
            nc.vector.tensor_copy(mmat[DE : DE + 1, :], sb_bo)

            # Two KV accumulators in separate PSUM banks: kvt_ps (chunks
            # 0..n-3) and kvb_ps (last two chunks).  Splitting KV lets
            # M = KV @ WO.T start before the final chunk arrives.  (Sharing
            # one bank between accumulation regions corrupts the results.)
            kvt_ps = kvps.tile([DE, DE], f32)
            kvb_ps = kvbps.tile([DE, DE], f32)

            # Keep-alive matmuls: the HAM clock gate re-throttles the PE to
            # 1.2 GHz after ~3.4us of low activity, which is exactly what the
            # KV -> M -> phase-C transition looks like (two small DVE copies
            # gate the next matmul).  A few discarded transposes keep the PE
            # stream dense so phase C runs at the warm 2.4 GHz.
            # warm-up operand with no DMA dependency: dummies can start right
            # after the engine preamble, ~6us before the first x chunk lands.
            warm_sb = consts.tile([128, 128], f16)
            nc.vector.memset(warm_sb, 0.0)

            # ---- Phase A: stream the whole batch, e2|e3 -> KV^T; e1 for
            # the own chunks (j < own_chunks) ----
            # Software-pipelined by one chunk so the Tensor engine's in-order
            # stream never blocks on the Vector/Scalar copies: per iteration
            # PE runs [transpose(j-1), e23T(j), KV(j-1), e1(j)] -- each op's
            # producer on the other engine finished an iteration ago.  Dense
            # PE occupancy also keeps the HAM clock gate at 2.4 GHz (a 50%
            # duty cycle PE stream runs at the cold 1.2 GHz forever).
            with (
                tc.tile_pool(name="xf", bufs=6) as xfp,
                tc.tile_pool(name="e23tps", bufs=3, space="PSUM") as e23tpsp,
                tc.tile_pool(name="e23tsb", bufs=4) as e23tsbp,
                tc.tile_pool(name="trps", bufs=2, space="PSUM") as trpsp,
                tc.tile_pool(name="e23n", bufs=3) as e23np,
                tc.tile_pool(name="e1ps", bufs=1, space="PSUM") as e1psp,
            ):
                # One bank, two partition-disjoint regions: e1 accumulates on
                # partitions 0..63, keep-alive dummies write partitions
                # 64..127.  (PSUM accumulation is per-address, but regions
                # must not overlap.)
                e1kv = e1psp.tile([128, 512], f32)

                def _dummy_mms(k):
                    for _ in range(k):
                        nc.tensor.matmul(
                            e1kv[DE:, :128], lhsT=warm_sb[:, :DE], rhs=warm_sb
                        )

                # HAM warm-up: ~5us of discarded matmuls fill the dead time
                # between the engine preamble and the first x chunk's arrival
                # so real matmuls start at the warm 2.4 GHz clock.  The junk
                # sigmoid loads the activation table up front.
                warm_o = small.tile([1, 4], f16)
                nc.scalar.activation(
                    warm_o, warm_sb[:1, :4], mybir.ActivationFunctionType.Sigmoid
                )
                _dummy_mms(24)
                state = {}  # j -> (e23t_sb | e23n) between pipeline stages

                def _transpose_part(j):
                    # transpose e23T back to natural layout (batched into one
                    # PSUM bank -> single DVE copy)
                    e23t_sb = state.pop(j)
                    tr_ps = trpsp.tile([128, 512], f16)
                    for t in range(4):
                        nc.tensor.transpose(
                            tr_ps[:, t * 128 : (t + 1) * 128],
                            e23t_sb[:, t * 128 : (t + 1) * 128],
                            identity[:, :],
                        )
                    e23n = e23np.tile([128, 512], f16)
                    nc.vector.tensor_copy(e23n, tr_ps)
                    state[j] = e23n

                def _kv_part(j):
                    # accumulate KV^T = e3^T @ e2 (the DVE copy of chunk j ran
                    # while the PE streamed chunk j+1's e23T -- no stall here).
                    # The last two chunks go to the second accumulator so the
                    # first (dominant) KV part can head into M early.
                    e23n = state.pop(j)
                    last2 = j >= n_chunks - 2
                    dst = kvb_ps if last2 else kvt_ps
                    lo = (n_chunks - 2) * 4 if last2 else 0
                    hi = 4 * n_chunks - 1 if last2 else (n_chunks - 2) * 4 - 1
                    for t in range(4):
                        tt = j * 4 + t
                        nc.tensor.matmul(
                            dst,
                            lhsT=e23n[:, t * 128 + DE : (t + 1) * 128],
                            rhs=e23n[:, t * 128 : t * 128 + DE],
                            start=(tt == lo),
                            stop=(tt == hi),
                        )

                def _front_half(j, xr):
                    # e23T = [W2;W3] @ x^T  -> [128, 512] (d on partitions)
                    e23t_ps = e23tpsp.tile([128, 512], f32)
                    for kt in range(4):
                        nc.tensor.matmul(
                            e23t_ps,
                            lhsT=sb_w23t[:, kt, :],
                            rhs=xr[:, kt, :],
                            start=(kt == 0),
                            stop=(kt == 3),
                        )
                    # bias add + f16 cast on DVE -- the Scalar engine stays
                    # on the Sigmoid table all kernel (a mid-kernel activation
                    # table switch costs 1.3us right in the tail).
                    e23t_sb = e23tsbp.tile([128, 512], f16)
                    nc.vector.tensor_scalar_add(e23t_sb, e23t_ps, sb_b23)
                    state[j] = e23t_sb

                    # e1T = W1 @ x^T (+b1) for the rows this core outputs.
                    # Own chunks ride FIRST: appending them last put their e1
                    # matmuls on the post-stream critical path (+5us).
                    if j < own_chunks:
                        jo = j
                        e1_ps = e1kv[:DE, :]
                        for kt in range(4):
                            nc.tensor.matmul(
                                e1_ps,
                                lhsT=sb_w1t[:, kt, :],
                                rhs=xr[:, kt, :],
                                start=(kt == 0),
                                stop=(kt == 3),
                            )
                        nc.vector.tensor_scalar_add(
                            e1t[:DE, jo * 512 : (jo + 1) * 512], e1_ps, sb_b1
                        )

                # Small leading DMAs get the first chunk on-chip (and the PE
                # warmed up) ~5us earlier than a uniform 4-chunk schedule;
                # the steady state stays at 16 KiB/partition per transfer.
                # The transpose/KV pass trails by TWO chunks: with a one-chunk
                # lag the transposes of j-1 stall ~0.7us per chunk waiting on
                # the bias engine (measured $S waits at every iteration start).
                schedule = [1, 1, 2] + [4] * ((n_chunks - 4) // 4)
                assert sum(schedule) == n_chunks
                # M = KV @ WO.T accumulates in the e1 region of the shared
                # bank -- e1 is long finished (own chunks come first).
                mm_ps = e1kv[:DE, :]
                j = 0
                for g in schedule:
                    xf = xfp.tile([128, 4, 2048], f16, tag="xf")
                    if j == 0:
                        # quarter-granularity leading DMAs: the first e23T
                        # matmul only needs the first 512 columns, so it can
                        # start ~3us before the whole chunk lands
                        for kt in range(4):
                            nc.sync.dma_start(
                                out=xf[:, 0, kt * 512 : (kt + 1) * 512],
                                in_=xt_t[:, 0, kt * 512 : (kt + 1) * 512],
                            )
                    else:
                        # alternate queue rings: if the ~290 GB/s sustained
                        # rate is a per-ring cap, two rings double it
                        eng = nc.sync if (j // 4) % 2 == 0 else nc.scalar
                        eng.dma_start(
                            out=xf[:, :g, :], in_=xt_t[:, j : j + g, :]
                        )
                    for j2 in range(g):
                        xr = xf[:, j2, :].rearrange("p (kt s) -> p kt s", kt=4)
                        if j > 0:
                            _transpose_part(j - 1)
                        _front_half(j, xr)
                        if j > 0:
                            _kv_part(j - 1)
                        if n_chunks - 4 <= j < n_chunks - 1:
                            # fill the last group's DMA-wait gaps so the HAM
                            # clock gate stays at 2.4 GHz into phase C
                            _dummy_mms(2)
                        if j == n_chunks - 1:
                            # chunks 0..n-3 are fully accumulated: start
                            # M = KV @ WO.T on the dominant part now.
                            kvt_r = small.tile([DE, DE], f16)
                            nc.vector.tensor_copy(kvt_r, kvt_ps)
                            nc.tensor.matmul(
                                mm_ps, lhsT=kvt_r, rhs=sb_wot,
                                start=True, stop=False,
                            )
                        j += 1
                _transpose_part(n_chunks - 1)
                _kv_part(n_chunks - 1)
                kvb_r = small.tile([DE, DE], f16)
                nc.vector.tensor_copy(kvb_r, kvb_ps)
                _dummy_mms(2)
                nc.tensor.matmul(
                    mm_ps, lhsT=kvb_r, rhs=sb_wot, start=False, stop=True
                )
                _dummy_mms(2)
                nc.vector.tensor_copy(mmat[:DE, :], mm_ps)

            # ---- Phase C: out = sigmoid(e1 @ M + bO) ----
            # The sigmoid is batched: DVE drains each PSUM tile to SBUF f16,
            # then ONE scalar ACTIVATE covers a whole 512-row chunk (2048
            # elems/partition) -- 4 big sigmoids instead of 16 small ones
            # (the per-instruction overhead and f32 input rate made 16
            # PSUM-sourced sigmoids a 13.6us serial tail on the Scalar
            # engine).
            # Graduated batch sizes: the first sigmoids cover 2 tiles so the
            # Scalar engine starts ~2us earlier; later ones cover 4 tiles to
            # amortize the per-instruction overhead.  The scalar chain is the
            # tail critical path (~1.2 el/ns/partition).
            with (
                tc.tile_pool(name="ops", bufs=3, space="PSUM") as opsp,
                tc.tile_pool(name="zsb", bufs=2) as zsbp,
                tc.tile_pool(name="osb", bufs=2) as osbp,
            ):
                def _dummy_c(k):
                    # phase-C keep-alives write the retired kvb bank
                    for _ in range(k):
                        nc.tensor.matmul(
                            kvb_ps, lhsT=warm_sb[:, :DE], rhs=warm_sb[:, :DE]
                        )

                out_flat = out.ap()
                for a, b in ((0, 1), (1, 2), (2, 4), (4, 8), (8, 14), (14, 16)):
                    osb = osbp.tile([128, b - a, DIN], f16, tag=f"o{b-a}")
                    direct = b - a == 1
                    zsb = (
                        None
                        if direct
                        else zsbp.tile([128, b - a, DIN], f16, tag=f"z{b-a}")
                    )
                    for t in range(a, b):
                        o_ps = opsp.tile([128, DIN], f32)
                        nc.tensor.matmul(
                            o_ps,
                            lhsT=e1t[: DE + 1, t * 128 : (t + 1) * 128],
                            rhs=mmat[: DE + 1, :],
                        )
                        if t < 6:
                            # densify the PE stream while the HAM clock gate
                            # decides whether phase C deserves 2.4 GHz
                            _dummy_c(1)
                        if direct:
                            # PSUM-direct sigmoid: no copy hop, starts the
                            # Scalar chain right after the first matmul
                            nc.scalar.activation(
                                osb[:, 0, :],
                                o_ps,
                                mybir.ActivationFunctionType.Sigmoid,
                            )
                        else:
                            nc.vector.tensor_copy(zsb[:, t - a, :], o_ps)
                    if not direct:
                        nc.scalar.activation(
                            osb,
                            zsb,
                            mybir.ActivationFunctionType.Sigmoid,
                        )
                    nc.sync.dma_start(
                        out=out_flat[:, a * DIN : b * DIN], in_=osb
                    )
    nc.compile()
    return nc


def make_wconst(W1, b1, W2, b2, W3, b3, WO, bO):
    blob = np.zeros((128, _NB), np.float16)
    w1t = np.asarray(W1, np.float16).T.reshape(4, 128, DE)  # (kt, p, d)
    blob[:, _OFF_W1T : _OFF_W1T + 256] = (
        w1t.transpose(1, 0, 2).reshape(128, 4 * DE)
    )
    w23t = np.concatenate(
        [np.asarray(W2, np.float16).T, np.asarray(W3, np.float16).T], axis=1
    ).reshape(4, 128, 2 * DE)
    blob[:, _OFF_W23T : _OFF_W23T + 512] = (
        w23t.transpose(1, 0, 2).reshape(128, 8 * DE)
    )
    blob[:, _OFF_IDENT : _OFF_IDENT + 128] = np.eye(128, dtype=np.float16)
    blob[:, _OFF_B23] = np.concatenate(
        [np.asarray(b2, np.float16), np.asarray(b3, np.float16)]
    )
    blob[:DE, _OFF_WOT : _OFF_WOT + DIN] = np.asarray(WO, np.float16).T
    blob[:DE, _OFF_B1] = np.asarray(b1, np.float16)
    blob[0, _OFF_BO : _OFF_BO + DIN] = np.asarray(bO, np.float16)
    return blob


def _tile_rows(xc):
    """[rows, 512] f16 -> [128, (rows/512)*2048] in (p, chunk, kt, s) order."""
    n = xc.shape[0] // 512
    return np.ascontiguousarray(
        xc.reshape(n, 512, 4, 128).transpose(3, 0, 2, 1)
    ).reshape(128, n * 2048)


def make_in_maps(x, W1, b1, W2, b2, W3, b3, WO, bO, rows=ROWS, n_cores=N_CORES):
    x = np.asarray(x, dtype=np.float32).astype(np.float16)
    total = x.shape[0] * x.shape[1]
    xf = x.reshape(total, DIN)
    blob = make_wconst(W1, b1, W2, b2, W3, b3, WO, bO)
    bvec = np.zeros((128, 2), np.float32)
    bvec[:, 0] = np.concatenate([np.asarray(b2, np.float32), np.asarray(b3, np.float32)])
    bvec[:DE, 1] = np.asarray(b1, np.float32)
    group = n_cores // 2
    batch_rows = rows * group
    in_maps = []
    for c in range(n_cores):
        b, q = divmod(c, group)
        xb = xf[b * batch_rows : (b + 1) * batch_rows]  # full batch of this core
        own = xb[q * rows : (q + 1) * rows]
        rest = np.concatenate([xb[: q * rows], xb[(q + 1) * rows :]], axis=0)
        m = {
            "wconst": blob,
            "bias32": bvec,
            "xt": np.concatenate([_tile_rows(own), _tile_rows(rest)], axis=1),
        }
        in_maps.append(m)
    return in_maps


def unshard_out(o, rows=ROWS):
    # o: [128, rows*4] f16 laid out (p, j, t, o) -> rows j*512 + t*128 + p
    n_chunks = rows // 512
    return (
        o.astype(np.float32)
        .reshape(128, n_chunks, 4, DIN)
        .transpose(1, 2, 0, 3)
        .reshape(rows, DIN)
    )


def kernel(x, W1, b1, W2, b2, W3, b3, WO, bO):
    global LAST_RESULT
    if "nc" not in _NC_CACHE:
        _NC_CACHE["nc"] = build_nc()
    nc = _NC_CACHE["nc"]
    in_maps = make_in_maps(x, W1, b1, W2, b2, W3, b3, WO, bO)
    res = run_bass_kernel_spmd(
        nc,
        in_maps,
        core_ids=list(range(N_CORES)),
        trace=TRACE,
        **TRACE_KWARGS,
    )
    LAST_RESULT = res
    full = np.concatenate(
        [unshard_out(res.results[c]["out"]) for c in range(N_CORES)], axis=0
    )  # [16384, 512] f32
    return full.reshape(BATCH, SEQ, DIN)


# revision 43
# speedup vs baseline: 1.1415x; 1.1415x over previous
# Trainium2 Bass kernel for nn_Attention3 (unnormalized linear attention).
#
# Math: e_i = x @ W_i.T + b_i (i=1,2,3);  out = sigmoid((e1 @ e2.T @ e3) @ WO.T + bO)
# Since there is no softmax, (e1 @ e2.T) @ e3 == e1 @ (e2.T @ e3) where
# KV = e2.T @ e3 is only [64, 64].
#
# Sharding: the flattened [B*S, 512] = [16384, 512] rows are split into 8
# chunks of 2048 rows (cores 0-3 <- batch 0, cores 4-7 <- batch 1).  Every
# core streams its WHOLE batch to build the full KV^T = e3.T @ e2 locally --
# redundant compute, but fully deterministic: no collectives or cross-core
# synchronization (measured ncfw AllGather latency on this setup is 25-100us
# with heavy per-core skew, far worse than the extra DMA).  Each core's OWN
# 2048 rows are ordered first in its input so e1 and the output stage run on
# chunks 0-3 with uniform (SPMD) code.
#
# Precision: x is cast to float16 on the HOST, halving HBM read traffic
# (16.8 -> 8.4 MB/core) and removing the on-chip f32->f16 DVE casts that
# were ~23us of Vector time.  All matmuls run f16 operands / fp32 PSUM.
# The output is written f16 (sigmoid range (0,1): ~5e-4 rounding) and
# upcast on the host.  Biases ride in the f16 weight blob.
#
# Layouts: x arrives host-transposed and pre-tiled as [128, chunk, 2048] f16
# so every DMA reads contiguous 4 KiB per partition (16 KiB per partition
# per 4-chunk group DMA); the output is written back partition-major
# ([128, rows*4] f16) and un-permuted on the host.  All weights/biases
# arrive packed in one [128, 1922] f16 blob (single DMA) and are used as
# views -- no unpack copies.

import numpy as np

import concourse.bass as bass
import concourse.mybir as mybir
import concourse.tile as tile
from concourse import bacc
from concourse.bass_utils import run_bass_kernel_spmd

BATCH = 2
SEQ = 8192
DIN = 512
DE = 64
N_CORES = 8
ROWS = (BATCH * SEQ) // N_CORES  # 2048 output rows per core

# const blob layout (free-dim offsets, f16, [128, NB])
_OFF_W1T = 0          # [128, 4, 64]   w1t rearranged (kt p) d -> p kt d
_OFF_W23T = 256       # [128, 4, 128]  w23t rearranged
_OFF_IDENT = 768      # [128, 128]     identity
_OFF_B23 = 896        # [128, 1]       b2|b3 (per-partition)
_OFF_WOT = 897        # [64, 512]      WO.T (rows 0..63)
_OFF_B1 = 1409        # [64, 1]        b1 (rows 0..63)
_OFF_BO = 1410        # [1, 512]       bO (row 0)
_NB = 1922

TRACE = False
TRACE_KWARGS = {}
LAST_RESULT = None

_NC_CACHE = {}


def build_nc(rows=ROWS, n_cores=N_CORES):
    f32 = mybir.dt.float32
    f16 = mybir.dt.float16

    group = n_cores // 2  # cores per batch
    assert rows % 512 == 0
    own_chunks = rows // 512
    n_chunks = own_chunks * group  # whole batch streamed per core

    nc = bacc.Bacc(
        None,
        target_bir_lowering=False,
        debug=False,
        num_devices=n_cores,
    )

    xt = nc.dram_tensor("xt", [128, n_chunks * 2048], f16, kind="ExternalInput")
    wconst = nc.dram_tensor("wconst", [128, _NB], f16, kind="ExternalInput")
    # b2|b3 (col 0, 128 rows) and b1 (col 1, rows 0..63) — DVE tensor_scalar
    # requires a float32 scalar operand, so these ride outside the f16 blob.
    bias32 = nc.dram_tensor("bias32", [128, 2], f32, kind="ExternalInput")
    out = nc.dram_tensor("out", [128, rows * 4], f16, kind="ExternalOutput")

    xt_t = xt.ap().rearrange("p (j f) -> p j f", f=2048)  # [128, n_chunks, 2048]
    out_t = out.ap().rearrange("p (j f) -> p j f", f=2048)

    with tile.TileContext(nc) as tc:
        with (
            tc.tile_pool(name="consts", bufs=1) as consts,
            tc.tile_pool(name="persist", bufs=1) as persist,
            tc.tile_pool(name="kvps", bufs=1, space="PSUM") as kvps,
            tc.tile_pool(name="kvbps", bufs=1, space="PSUM") as kvbps,
            tc.tile_pool(name="small", bufs=1) as small,
        ):
            blob = consts.tile([128, _NB], f16)
            nc.sync.dma_start(out=blob, in_=wconst.ap())
            biases = consts.tile([128, 2], f32)
            nc.sync.dma_start(out=biases, in_=bias32.ap())

            sb_w1t = blob[:, _OFF_W1T : _OFF_W1T + 256].rearrange(
                "p (kt d) -> p kt d", kt=4
            )
            sb_w23t = blob[:, _OFF_W23T : _OFF_W23T + 512].rearrange(
                "p (kt d) -> p kt d", kt=4
            )
            sb_wot = blob[:DE, _OFF_WOT : _OFF_WOT + DIN]
            identity = blob[:, _OFF_IDENT : _OFF_IDENT + 128]
            sb_b23 = biases[:, 0:1]
            sb_b1 = biases[:DE, 1:2]
            sb_bo = blob[:1, _OFF_BO : _OFF_BO + DIN]

            # e1^T for the own rows, with a row of ones at partition DE so the
            # final matmul folds in the output bias (lhsT K = DE+1).
            e1t = persist.tile([128, rows], f16)
            nc.vector.memset(e1t[DE : DE + 1, :], 1.0)
            # M = KV @ WO.T in rows 0..63, bO in row DE.
            mmat = persist.tile([128, DIN], f16)
            nc.vector.tensor_copy(mmat[DE : DE + 1, :], sb_bo)

            # Two KV accumulators in separate PSUM banks: kvt_ps (chunks
            # 0..n-3) and kvb_ps (last two chunks).  Splitting KV lets
            # M = KV @ WO.T start before the final chunk arrives.  (Sharing
            # one bank between accumulation regions corrupts the results.)
            kvt_ps = kvps.tile([DE, DE], f32)
            kvb_ps = kvbps.tile([DE, DE], f32)

            # Keep-alive matmuls: the HAM clock gate re-throttles the PE to
            # 1.2 GHz after ~3.4us of low activity, which is exactly what the
            # KV -> M -> phase-C transition looks like (two small DVE copies
            # gate the next matmul).  A few discarded transposes keep the PE
            # stream dense so phase C runs at the warm 2.4 GHz.
            # warm-up operand with no DMA dependency: dummies can start right
            # after the engine preamble, ~6us before the first x chunk lands.
            warm_sb = consts.tile([128, 128], f16)
            nc.vector.memset(warm_sb, 0.0)

            # ---- Phase A: stream the whole batch, e2|e3 -> KV^T; e1 for
            # the own chunks (j < own_chunks) ----
            # Software-pipelined by one chunk so the Tensor engine's in-order
            # stream never blocks on the Vector/Scalar copies: per iteration
            # PE runs [transpose(j-1), e23T(j), KV(j-1), e1(j)] -- each op's
            # producer on the other engine finished an iteration ago.  Dense
            # PE occupancy also keeps the HAM clock gate at 2.4 GHz (a 50%
            # duty cycle PE stream runs at the cold 1.2 GHz forever).
            with (
                tc.tile_pool(name="xf", bufs=6) as xfp,
                tc.tile_pool(name="e23tps", bufs=3, space="PSUM") as e23tpsp,
                tc.tile_pool(name="e23tsb", bufs=4) as e23tsbp,
                tc.tile_pool(name="trps", bufs=2, space="PSUM") as trpsp,
                tc.tile_pool(name="e23n", bufs=3) as e23np,
                tc.tile_pool(name="e1ps", bufs=1, space="PSUM") as e1psp,
            ):
                # One bank, two partition-disjoint regions: e1 accumulates on
                # partitions 0..63, keep-alive dummies write partitions
                # 64..127.  (PSUM accumulation is per-address, but regions
                # must not overlap.)
                e1kv = e1psp.tile([128, 512], f32)

                def _dummy_mms(k):
                    for _ in range(k):
                        nc.tensor.matmul(
                            e1kv[DE:, :128], lhsT=warm_sb[:, :DE], rhs=warm_sb
                        )

                # HAM warm-up: ~5us of discarded matmuls fill the dead time
                # between the engine preamble and the first x chunk's arrival
                # so real matmuls start at the warm 2.4 GHz clock.  The junk
                # sigmoid loads the activation table up front.
                warm_o = small.tile([1, 4], f16)
                nc.scalar.activation(
                    warm_o, warm_sb[:1, :4], mybir.ActivationFunctionType.Sigmoid
                )
                _dummy_mms(24)
                state = {}  # j -> (e23t_sb | e23n) between pipeline stages

                def _transpose_part(j):
                    # transpose e23T back to natural layout (batched into one
                    # PSUM bank -> single DVE copy)
                    e23t_sb = state.pop(j)
                    tr_ps = trpsp.tile([128, 512], f16)
                    for t in range(4):
                        nc.tensor.transpose(
                            tr_ps[:, t * 128 : (t + 1) * 128],
                            e23t_sb[:, t * 128 : (t + 1) * 128],
                            identity[:, :],
                        )
                    e23n = e23np.tile([128, 512], f16)
                    nc.vector.tensor_copy(e23n, tr_ps)
                    state[j] = e23n

                def _kv_part(j):
                    # accumulate KV^T = e3^T @ e2 (the DVE copy of chunk j ran
                    # while the PE streamed chunk j+1's e23T -- no stall here).
                    # The last two chunks go to the second accumulator so the
                    # first (dominant) KV part can head into M early.
                    e23n = state.pop(j)
                    last2 = j >= n_chunks - 2
                    dst = kvb_ps if last2 else kvt_ps
                    lo = (n_chunks - 2) * 4 if last2 else 0
                    hi = 4 * n_chunks - 1 if last2 else (n_chunks - 2) * 4 - 1
                    for t in range(4):
                        tt = j * 4 + t
                        nc.tensor.matmul(
                            dst,
                            lhsT=e23n[:, t * 128 + DE : (t + 1) * 128],
                            rhs=e23n[:, t * 128 : t * 128 + DE],
                            start=(tt == lo),
                            stop=(tt == hi),
                        )

                def _front_half(j, xr):
                    # e23T = [W2;W3] @ x^T  -> [128, 512] (d on partitions)
                    e23t_ps = e23tpsp.tile([128, 512], f32)
                    for kt in range(4):
                        nc.tensor.matmul(
                            e23t_ps,
                            lhsT=sb_w23t[:, kt, :],
                            rhs=xr[:, kt, :],
                            start=(kt == 0),
                            stop=(kt == 3),
                        )
                    # bias add + f16 cast on DVE -- the Scalar engine stays
                    # on the Sigmoid table all kernel (a mid-kernel activation
                    # table switch costs 1.3us right in the tail).
                    e23t_sb = e23tsbp.tile([128, 512], f16)
                    nc.vector.tensor_scalar_add(e23t_sb, e23t_ps, sb_b23)
                    state[j] = e23t_sb

                    # e1T = W1 @ x^T (+b1) for the rows this core outputs.
                    # Own chunks ride FIRST: appending them last put their e1
                    # matmuls on the post-stream critical path (+5us).
                    if j < own_chunks:
                        jo = j
                        e1_ps = e1kv[:DE, :]
                        for kt in range(4):
                            nc.tensor.matmul(
                                e1_ps,
                                lhsT=sb_w1t[:, kt, :],
                                rhs=xr[:, kt, :],
                                start=(kt == 0),
                                stop=(kt == 3),
                            )
                        nc.vector.tensor_scalar_add(
                            e1t[:DE, jo * 512 : (jo + 1) * 512], e1_ps, sb_b1
                        )

                # Small leading DMAs get the first chunk on-chip (and the PE
                # warmed up) ~5us earlier than a uniform 4-chunk schedule;
                # the steady state stays at 16 KiB/partition per transfer.
                # The transpose/KV pass trails by TWO chunks: with a one-chunk
                # lag the transposes of j-1 stall ~0.7us per chunk waiting on
                # the bias engine (measured $S waits at every iteration start).
                schedule = [1, 1, 2] + [4] * ((n_chunks - 4) // 4)
                assert sum(schedule) == n_chunks
                # M = KV @ WO.T accumulates in the e1 region of the shared
                # bank -- e1 is long finished (own chunks come first).
                mm_ps = e1kv[:DE, :]
                j = 0
                for g in schedule:
                    xf = xfp.tile([128, 4, 2048], f16, tag="xf")
                    if j == 0:
                        # quarter-granularity leading DMAs: the first e23T
                        # matmul only needs the first 512 columns, so it can
                        # start ~3us before the whole chunk lands
                        for kt in range(4):
                            nc.sync.dma_start(
                                out=xf[:, 0, kt * 512 : (kt + 1) * 512],
                                in_=xt_t[:, 0, kt * 512 : (kt + 1) * 512],
                            )
                    else:
                        nc.sync.dma_start(
                            out=xf[:, :g, :], in_=xt_t[:, j : j + g, :]
                        )
                    for j2 in range(g):
                        xr = xf[:, j2, :].rearrange("p (kt s) -> p kt s", kt=4)
                        if j > 0:
                            _transpose_part(j - 1)
                        _front_half(j, xr)
                        if j > 0:
                            _kv_part(j - 1)
                        if n_chunks - 4 <= j < n_chunks - 1:
                            # fill the last group's DMA-wait gaps so the HAM
                            # clock gate stays at 2.4 GHz into phase C
                            _dummy_mms(2)
                        if j == n_chunks - 1:
                            # chunks 0..n-3 are fully accumulated: start
                            # M = KV @ WO.T on the dominant part now.
                            kvt_r = small.tile([DE, DE], f16)
                            nc.vector.tensor_copy(kvt_r, kvt_ps)
                            nc.tensor.matmul(
                                mm_ps, lhsT=kvt_r, rhs=sb_wot,
                                start=True, stop=False,
                            )
                        j += 1
                _transpose_part(n_chunks - 1)
                _kv_part(n_chunks - 1)
                kvb_r = small.tile([DE, DE], f16)
                nc.vector.tensor_copy(kvb_r, kvb_ps)
                _dummy_mms(2)
                nc.tensor.matmul(
                    mm_ps, lhsT=kvb_r, rhs=sb_wot, start=False, stop=True
                )
                _dummy_mms(2)
                nc.vector.tensor_copy(mmat[:DE, :], mm_ps)

            # ---- Phase C: out = sigmoid(e1 @ M + bO) ----
            # The sigmoid is batched: DVE drains each PSUM tile to SBUF f16,
            # then ONE scalar ACTIVATE covers a whole 512-row chunk (2048
            # elems/partition) -- 4 big sigmoids instead of 16 small ones
            # (the per-instruction overhead and f32 input rate made 16
            # PSUM-sourced sigmoids a 13.6us serial tail on the Scalar
            # engine).
            # Graduated batch sizes: the first sigmoids cover 2 tiles so the
            # Scalar engine starts ~2us earlier; later ones cover 4 tiles to
            # amortize the per-instruction overhead.  The scalar chain is the
            # tail critical path (~1.2 el/ns/partition).
            with (
                tc.tile_pool(name="ops", bufs=3, space="PSUM") as opsp,
                tc.tile_pool(name="zsb", bufs=2) as zsbp,
                tc.tile_pool(name="osb", bufs=2) as osbp,
            ):
                def _dummy_c(k):
                    # phase-C keep-alives write the retired kvb bank
                    for _ in range(k):
                        nc.tensor.matmul(
                            kvb_ps, lhsT=warm_sb[:, :DE], rhs=warm_sb[:, :DE]
                        )

                out_flat = out.ap()
                for a, b in ((0, 1), (1, 2), (2, 4), (4, 8), (8, 14), (14, 16)):
                    osb = osbp.tile([128, b - a, DIN], f16, tag=f"o{b-a}")
                    direct = b - a == 1
                    zsb = (
                        None
                        if direct
                        else zsbp.tile([128, b - a, DIN], f16, tag=f"z{b-a}")
                    )
                    for t in range(a, b):
                        o_ps = opsp.tile([128, DIN], f32)
                        nc.tensor.matmul(
                            o_ps,
                            lhsT=e1t[: DE + 1, t * 128 : (t + 1) * 128],
                            rhs=mmat[: DE + 1, :],
                        )
                        if t < 6:
                            # densify the PE stream while the HAM clock gate
                            # decides whether phase C deserves 2.4 GHz
                            _dummy_c(1)
                        if direct:
                            # PSUM-direct sigmoid: no copy hop, starts the
                            # Scalar chain right after the first matmul
                            nc.scalar.activation(
                                osb[:, 0, :],
                                o_ps,
                                mybir.ActivationFunctionType.Sigmoid,
                            )
                        else:
                            nc.vector.tensor_copy(zsb[:, t - a, :], o_ps)
                    if not direct:
                        nc.scalar.activation(
                            osb,
                            zsb,
                            mybir.ActivationFunctionType.Sigmoid,
                        )
                    nc.sync.dma_start(
                        out=out_flat[:, a * DIN : b * DIN], in_=osb
                    )
    nc.compile()
    return nc


def make_wconst(W1, b1, W2, b2, W3, b3, WO, bO):
    blob = np.zeros((128, _NB), np.float16)
    w1t = np.asarray(W1, np.float16).T.reshape(4, 128, DE)  # (kt, p, d)
    blob[:, _OFF_W1T : _OFF_W1T + 256] = (
        w1t.transpose(1, 0, 2).reshape(128, 4 * DE)
    )
    w23t = np.concatenate(
        [np.asarray(W2, np.float16).T, np.asarray(W3, np.float16).T], axis=1
    ).reshape(4, 128, 2 * DE)
    blob[:, _OFF_W23T : _OFF_W23T + 512] = (
        w23t.transpose(1, 0, 2).reshape(128, 8 * DE)
    )
    blob[:, _OFF_IDENT : _OFF_IDENT + 128] = np.eye(128, dtype=np.float16)
    blob[:, _OFF_B23] = np.concatenate(
        [np.asarray(b2, np.float16), np.asarray(b3, np.float16)]
    )
    blob[:DE, _OFF_WOT : _OFF_WOT + DIN] = np.asarray(WO, np.float16).T
    blob[:DE, _OFF_B1] = np.asarray(b1, np.float16)
    blob[0, _OFF_BO : _OFF_BO + DIN] = np.asarray(bO, np.float16)
    return blob


def _tile_rows(xc):
    """[rows, 512] f16 -> [128, (rows/512)*2048] in (p, chunk, kt, s) order."""
    n = xc.shape[0] // 512
    return np.ascontiguousarray(
        xc.reshape(n, 512, 4, 128).transpose(3, 0, 2, 1)
    ).reshape(128, n * 2048)


def make_in_maps(x, W1, b1, W2, b2, W3, b3, WO, bO, rows=ROWS, n_cores=N_CORES):
    x = np.asarray(x, dtype=np.float32).astype(np.float16)
    total = x.shape[0] * x.shape[1]
    xf = x.reshape(total, DIN)
    blob = make_wconst(W1, b1, W2, b2, W3, b3, WO, bO)
    bvec = np.zeros((128, 2), np.float32)
    bvec[:, 0] = np.concatenate([np.asarray(b2, np.float32), np.asarray(b3, np.float32)])
    bvec[:DE, 1] = np.asarray(b1, np.float32)
    group = n_cores // 2
    batch_rows = rows * group
    in_maps = []
    for c in range(n_cores):
        b, q = divmod(c, group)
        xb = xf[b * batch_rows : (b + 1) * batch_rows]  # full batch of this core
        own = xb[q * rows : (q + 1) * rows]
        rest = np.concatenate([xb[: q * rows], xb[(q + 1) * rows :]], axis=0)
        m = {
            "wconst": blob,
            "bias32": bvec,
            "xt": np.concatenate([_tile_rows(own), _tile_rows(rest)], axis=1),
        }
        in_maps.append(m)
    return in_maps


def unshard_out(o, rows=ROWS):
    # o: [128, rows*4] f16 laid out (p, j, t, o) -> rows j*512 + t*128 + p
    n_chunks = rows // 512
    return (
        o.astype(np.float32)
        .reshape(128, n_chunks, 4, DIN)
        .transpose(1, 2, 0, 3)
        .reshape(rows, DIN)
    )


def kernel(x, W1, b1, W2, b2, W3, b3, WO, bO):
    global LAST_RESULT
    if "nc" not in _NC_CACHE:
        _NC_CACHE["nc"] = build_nc()
    nc = _NC_CACHE["nc"]
    in_maps = make_in_maps(x, W1, b1, W2, b2, W3, b3, WO, bO)
    res = run_bass_kernel_spmd(
        nc,
        in_maps,
        core_ids=list(range(N_CORES)),
        trace=TRACE,
        **TRACE_KWARGS,
    )
    LAST_RESULT = res
    full = np.concatenate(
        [unshard_out(res.results[c]["out"]) for c in range(N_CORES)], axis=0
    )  # [16384, 512] f32
    return full.reshape(BATCH, SEQ, DIN)


# revision 44
# speedup vs baseline: 1.1447x; 1.0028x over previous
# Trainium2 Bass kernel for nn_Attention3 (unnormalized linear attention).
#
# Math: e_i = x @ W_i.T + b_i (i=1,2,3);  out = sigmoid((e1 @ e2.T @ e3) @ WO.T + bO)
# Since there is no softmax, (e1 @ e2.T) @ e3 == e1 @ (e2.T @ e3) where
# KV = e2.T @ e3 is only [64, 64].
#
# Sharding: the flattened [B*S, 512] = [16384, 512] rows are split into 8
# chunks of 2048 rows (cores 0-3 <- batch 0, cores 4-7 <- batch 1).  Every
# core streams its WHOLE batch to build the full KV^T = e3.T @ e2 locally --
# redundant compute, but fully deterministic: no collectives or cross-core
# synchronization (measured ncfw AllGather latency on this setup is 25-100us
# with heavy per-core skew, far worse than the extra DMA).  Each core's OWN
# 2048 rows are ordered first in its input so e1 and the output stage run on
# chunks 0-3 with uniform (SPMD) code.
#
# Precision: x is cast to float16 on the HOST, halving HBM read traffic
# (16.8 -> 8.4 MB/core) and removing the on-chip f32->f16 DVE casts that
# were ~23us of Vector time.  All matmuls run f16 operands / fp32 PSUM.
# The output is written f16 (sigmoid range (0,1): ~5e-4 rounding) and
# upcast on the host.  Biases ride in the f16 weight blob.
#
# Layouts: x arrives host-transposed and pre-tiled as [128, chunk, 2048] f16
# so every DMA reads contiguous 4 KiB per partition (16 KiB per partition
# per 4-chunk group DMA); the output is written back partition-major
# ([128, rows*4] f16) and un-permuted on the host.  All weights/biases
# arrive packed in one [128, 1922] f16 blob (single DMA) and are used as
# views -- no unpack copies.

import numpy as np

import concourse.bass as bass
import concourse.mybir as mybir
import concourse.tile as tile
from concourse import bacc
from concourse.bass_utils import run_bass_kernel_spmd

BATCH = 2
SEQ = 8192
DIN = 512
DE = 64
N_CORES = 8
ROWS = (BATCH * SEQ) // N_CORES  # 2048 output rows per core

# const blob layout (free-dim offsets, f16, [128, NB])
_OFF_W1T = 0          # [128, 4, 64]   w1t rearranged (kt p) d -> p kt d
_OFF_W23T = 256       # [128, 4, 128]  w23t rearranged
_OFF_IDENT = 768      # [128, 128]     identity
_OFF_B23 = 896        # [128, 1]       b2|b3 (per-partition)
_OFF_WOT = 897        # [64, 512]      WO.T (rows 0..63)
_OFF_B1 = 1409        # [64, 1]        b1 (rows 0..63)
_OFF_BO = 1410        # [1, 512]       bO (row 0)
_NB = 1922

TRACE = False
TRACE_KWARGS = {}
LAST_RESULT = None

_NC_CACHE = {}


def build_nc(rows=ROWS, n_cores=N_CORES):
    f32 = mybir.dt.float32
    f16 = mybir.dt.float16

    group = n_cores // 2  # cores per batch
    assert rows % 512 == 0
    own_chunks = rows // 512
    n_chunks = own_chunks * group  # whole batch streamed per core

    nc = bacc.Bacc(
        None,
        target_bir_lowering=False,
        debug=False,
        num_devices=n_cores,
    )

    xt = nc.dram_tensor("xt", [128, n_chunks * 2048], f16, kind="ExternalInput")
    wconst = nc.dram_tensor("wconst", [128, _NB], f16, kind="ExternalInput")
    # b2|b3 (col 0, 128 rows) and b1 (col 1, rows 0..63) — DVE tensor_scalar
    # requires a float32 scalar operand, so these ride outside the f16 blob.
    bias32 = nc.dram_tensor("bias32", [128, 2], f32, kind="ExternalInput")
    out = nc.dram_tensor("out", [128, rows * 4], f16, kind="ExternalOutput")

    xt_t = xt.ap().rearrange("p (j f) -> p j f", f=2048)  # [128, n_chunks, 2048]
    out_t = out.ap().rearrange("p (j f) -> p j f", f=2048)

    with tile.TileContext(nc) as tc:
        with (
            tc.tile_pool(name="consts", bufs=1) as consts,
            tc.tile_pool(name="persist", bufs=1) as persist,
            tc.tile_pool(name="kvps", bufs=1, space="PSUM") as kvps,
            tc.tile_pool(name="kvbps", bufs=1, space="PSUM") as kvbps,
            tc.tile_pool(name="small", bufs=1) as small,
        ):
            blob = consts.tile([128, _NB], f16)
            nc.sync.dma_start(out=blob, in_=wconst.ap())
            biases = consts.tile([128, 2], f32)
            nc.sync.dma_start(out=biases, in_=bias32.ap())

            sb_w1t = blob[:, _OFF_W1T : _OFF_W1T + 256].rearrange(
                "p (kt d) -> p kt d", kt=4
            )
            sb_w23t = blob[:, _OFF_W23T : _OFF_W23T + 512].rearrange(
                "p (kt d) -> p kt d", kt=4
            )
            sb_wot = blob[:DE, _OFF_WOT : _OFF_WOT + DIN]
            identity = blob[:, _OFF_IDENT : _OFF_IDENT + 128]
            sb_b23 = biases[:, 0:1]
            sb_b1 = biases[:DE, 1:2]
            sb_bo = blob[:1, _OFF_BO : _OFF_BO + DIN]

            # e1^T for the own rows, with a row of ones at partition DE so the
            # final matmul folds in the output bias (lhsT K = DE+1).
            e1t = persist.tile([128, rows], f16)
            nc.vector.memset(e1t[DE : DE + 1, :], 1.0)
            # M = KV @ WO.T in rows 0..63, bO in row DE.
            mmat = persist.tile([128, DIN], f16)
            nc.vector.tensor_copy(mmat[DE : DE + 1, :], sb_bo)

            # Two KV accumulators in separate PSUM banks: kvt_ps (chunks
            # 0..n-3) and kvb_ps (last two chunks).  Splitting KV lets
            # M = KV @ WO.T start before the final chunk arrives.  (Sharing
            # one bank between accumulation regions corrupts the results.)
            kvt_ps = kvps.tile([DE, DE], f32)
            kvb_ps = kvbps.tile([DE, DE], f32)

            # Keep-alive matmuls: the HAM clock gate re-throttles the PE to
            # 1.2 GHz after ~3.4us of low activity, which is exactly what the
            # KV -> M -> phase-C transition looks like (two small DVE copies
            # gate the next matmul).  A few discarded transposes keep the PE
            # stream dense so phase C runs at the warm 2.4 GHz.
            # warm-up operand with no DMA dependency: dummies can start right
            # after the engine preamble, ~6us before the first x chunk lands.
            warm_sb = consts.tile([128, 128], f16)
            nc.vector.memset(warm_sb, 0.0)

            # ---- Phase A: stream the whole batch, e2|e3 -> KV^T; e1 for
            # the own chunks (j < own_chunks) ----
            # Software-pipelined by one chunk so the Tensor engine's in-order
            # stream never blocks on the Vector/Scalar copies: per iteration
            # PE runs [transpose(j-1), e23T(j), KV(j-1), e1(j)] -- each op's
            # producer on the other engine finished an iteration ago.  Dense
            # PE occupancy also keeps the HAM clock gate at 2.4 GHz (a 50%
            # duty cycle PE stream runs at the cold 1.2 GHz forever).
            with (
                tc.tile_pool(name="xf", bufs=6) as xfp,
                tc.tile_pool(name="e23tps", bufs=3, space="PSUM") as e23tpsp,
                tc.tile_pool(name="e23tsb", bufs=4) as e23tsbp,
                tc.tile_pool(name="trps", bufs=2, space="PSUM") as trpsp,
                tc.tile_pool(name="e23n", bufs=3) as e23np,
                tc.tile_pool(name="e1ps", bufs=1, space="PSUM") as e1psp,
            ):
                # One bank, two partition-disjoint regions: e1 accumulates on
                # partitions 0..63, keep-alive dummies write partitions
                # 64..127.  (PSUM accumulation is per-address, but regions
                # must not overlap.)
                e1kv = e1psp.tile([128, 512], f32)

                def _dummy_mms(k):
                    for _ in range(k):
                        nc.tensor.matmul(
                            e1kv[DE:, :128], lhsT=warm_sb[:, :DE], rhs=warm_sb
                        )

                # HAM warm-up: ~5us of discarded matmuls fill the dead time
                # between the engine preamble and the first x chunk's arrival
                # so real matmuls start at the warm 2.4 GHz clock.  The junk
                # sigmoid loads the activation table up front.
                warm_o = small.tile([1, 4], f16)
                nc.scalar.activation(
                    warm_o, warm_sb[:1, :4], mybir.ActivationFunctionType.Sigmoid
                )
                _dummy_mms(24)
                state = {}  # j -> (e23t_sb | e23n) between pipeline stages

                def _transpose_part(j):
                    # transpose e23T back to natural layout (batched into one
                    # PSUM bank -> single DVE copy)
                    e23t_sb = state.pop(j)
                    tr_ps = trpsp.tile([128, 512], f16)
                    for t in range(4):
                        nc.tensor.transpose(
                            tr_ps[:, t * 128 : (t + 1) * 128],
                            e23t_sb[:, t * 128 : (t + 1) * 128],
                            identity[:, :],
                        )
                    e23n = e23np.tile([128, 512], f16)
                    nc.vector.tensor_copy(e23n, tr_ps)
                    state[j] = e23n

                def _kv_part(j):
                    # accumulate KV^T = e3^T @ e2 (the DVE copy of chunk j ran
                    # while the PE streamed chunk j+1's e23T -- no stall here).
                    # The last two chunks go to the second accumulator so the
                    # first (dominant) KV part can head into M early.
                    e23n = state.pop(j)
                    last2 = j >= n_chunks - 2
                    dst = kvb_ps if last2 else kvt_ps
                    lo = (n_chunks - 2) * 4 if last2 else 0
                    hi = 4 * n_chunks - 1 if last2 else (n_chunks - 2) * 4 - 1
                    for t in range(4):
                        tt = j * 4 + t
                        nc.tensor.matmul(
                            dst,
                            lhsT=e23n[:, t * 128 + DE : (t + 1) * 128],
                            rhs=e23n[:, t * 128 : t * 128 + DE],
                            start=(tt == lo),
                            stop=(tt == hi),
                        )

                def _front_half(j, xr):
                    # e23T = [W2;W3] @ x^T  -> [128, 512] (d on partitions)
                    e23t_ps = e23tpsp.tile([128, 512], f32)
                    for kt in range(4):
                        nc.tensor.matmul(
                            e23t_ps,
                            lhsT=sb_w23t[:, kt, :],
                            rhs=xr[:, kt, :],
                            start=(kt == 0),
                            stop=(kt == 3),
                        )
                    # bias add + f16 cast on DVE -- the Scalar engine stays
                    # on the Sigmoid table all kernel (a mid-kernel activation
                    # table switch costs 1.3us right in the tail).
                    e23t_sb = e23tsbp.tile([128, 512], f16)
                    nc.vector.tensor_scalar_add(e23t_sb, e23t_ps, sb_b23)
                    state[j] = e23t_sb

                    # e1T = W1 @ x^T (+b1) for the rows this core outputs.
                    # Own chunks ride FIRST: appending them last put their e1
                    # matmuls on the post-stream critical path (+5us).
                    if j < own_chunks:
                        jo = j
                        e1_ps = e1kv[:DE, :]
                        for kt in range(4):
                            nc.tensor.matmul(
                                e1_ps,
                                lhsT=sb_w1t[:, kt, :],
                                rhs=xr[:, kt, :],
                                start=(kt == 0),
                                stop=(kt == 3),
                            )
                        nc.vector.tensor_scalar_add(
                            e1t[:DE, jo * 512 : (jo + 1) * 512], e1_ps, sb_b1
                        )

                # Small leading DMAs get the first chunk on-chip (and the PE
                # warmed up) ~5us earlier than a uniform 4-chunk schedule;
                # the steady state stays at 16 KiB/partition per transfer.
                # The transpose/KV pass trails by TWO chunks: with a one-chunk
                # lag the transposes of j-1 stall ~0.7us per chunk waiting on
                # the bias engine (measured $S waits at every iteration start).
                schedule = [1, 1, 2] + [4] * ((n_chunks - 4) // 4)
                assert sum(schedule) == n_chunks
                # M = KV @ WO.T accumulates in the e1 region of the shared
                # bank -- e1 is long finished (own chunks come first).
                mm_ps = e1kv[:DE, :]
                j = 0
                for g in schedule:
                    xf = xfp.tile([128, 4, 2048], f16, tag="xf")
                    if j == 0:
                        # quarter-granularity leading DMAs: the first e23T
                        # matmul only needs the first 512 columns, so it can
                        # start ~3us before the whole chunk lands
                        for kt in range(4):
                            nc.sync.dma_start(
                                out=xf[:, 0, kt * 512 : (kt + 1) * 512],
                                in_=xt_t[:, 0, kt * 512 : (kt + 1) * 512],
                            )
                    else:
                        nc.sync.dma_start(
                            out=xf[:, :g, :], in_=xt_t[:, j : j + g, :]
                        )
                    for j2 in range(g):
                        xr = xf[:, j2, :].rearrange("p (kt s) -> p kt s", kt=4)
                        if j > 0:
                            _transpose_part(j - 1)
                        _front_half(j, xr)
                        if j > 0:
                            _kv_part(j - 1)
                        if n_chunks - 4 <= j < n_chunks - 1:
                            # fill the last group's DMA-wait gaps so the HAM
                            # clock gate stays at 2.4 GHz into phase C
                            _dummy_mms(2)
                        if j == n_chunks - 1:
                            # chunks 0..n-3 are fully accumulated: start
                            # M = KV @ WO.T on the dominant part now.
                            kvt_r = small.tile([DE, DE], f16)
                            nc.vector.tensor_copy(kvt_r, kvt_ps)
                            nc.tensor.matmul(
                                mm_ps, lhsT=kvt_r, rhs=sb_wot,
                                start=True, stop=False,
                            )
                        j += 1
                _transpose_part(n_chunks - 1)
                _kv_part(n_chunks - 1)
                kvb_r = small.tile([DE, DE], f16)
                nc.vector.tensor_copy(kvb_r, kvb_ps)
                _dummy_mms(2)
                nc.tensor.matmul(
                    mm_ps, lhsT=kvb_r, rhs=sb_wot, start=False, stop=True
                )
                _dummy_mms(2)
                nc.vector.tensor_copy(mmat[:DE, :], mm_ps)

            # ---- Phase C: out = sigmoid(e1 @ M + bO) ----
            # The sigmoid is batched: DVE drains each PSUM tile to SBUF f16,
            # then ONE scalar ACTIVATE covers a whole 512-row chunk (2048
            # elems/partition) -- 4 big sigmoids instead of 16 small ones
            # (the per-instruction overhead and f32 input rate made 16
            # PSUM-sourced sigmoids a 13.6us serial tail on the Scalar
            # engine).
            # Graduated batch sizes: the first sigmoids cover 2 tiles so the
            # Scalar engine starts ~2us earlier; later ones cover 4 tiles to
            # amortize the per-instruction overhead.  The scalar chain is the
            # tail critical path (~1.2 el/ns/partition).
            with (
                tc.tile_pool(name="ops", bufs=3, space="PSUM") as opsp,
                tc.tile_pool(name="zsb", bufs=2) as zsbp,
                tc.tile_pool(name="osb", bufs=2) as osbp,
            ):
                out_flat = out.ap()
                for a, b in ((0, 1), (1, 2), (2, 4), (4, 8), (8, 14), (14, 16)):
                    osb = osbp.tile([128, b - a, DIN], f16, tag=f"o{b-a}")
                    direct = b - a == 1
                    zsb = (
                        None
                        if direct
                        else zsbp.tile([128, b - a, DIN], f16, tag=f"z{b-a}")
                    )
                    for t in range(a, b):
                        o_ps = opsp.tile([128, DIN], f32)
                        nc.tensor.matmul(
                            o_ps,
                            lhsT=e1t[: DE + 1, t * 128 : (t + 1) * 128],
                            rhs=mmat[: DE + 1, :],
                        )
                        if direct:
                            # PSUM-direct sigmoid: no copy hop, starts the
                            # Scalar chain right after the first matmul
                            nc.scalar.activation(
                                osb[:, 0, :],
                                o_ps,
                                mybir.ActivationFunctionType.Sigmoid,
                            )
                        else:
                            nc.vector.tensor_copy(zsb[:, t - a, :], o_ps)
                    if not direct:
                        nc.scalar.activation(
                            osb,
                            zsb,
                            mybir.ActivationFunctionType.Sigmoid,
                        )
                    nc.sync.dma_start(
                        out=out_flat[:, a * DIN : b * DIN], in_=osb
                    )
    nc.compile()
    return nc


def make_wconst(W1, b1, W2, b2, W3, b3, WO, bO):
    blob = np.zeros((128, _NB), np.float16)
    w1t = np.asarray(W1, np.float16).T.reshape(4, 128, DE)  # (kt, p, d)
    blob[:, _OFF_W1T : _OFF_W1T + 256] = (
        w1t.transpose(1, 0, 2).reshape(128, 4 * DE)
    )
    w23t = np.concatenate(
        [np.asarray(W2, np.float16).T, np.asarray(W3, np.float16).T], axis=1
    ).reshape(4, 128, 2 * DE)
    blob[:, _OFF_W23T : _OFF_W23T + 512] = (
        w23t.transpose(1, 0, 2).reshape(128, 8 * DE)
    )
    blob[:, _OFF_IDENT : _OFF_IDENT + 128] = np.eye(128, dtype=np.float16)
    blob[:, _OFF_B23] = np.concatenate(
        [np.asarray(b2, np.float16), np.asarray(b3, np.float16)]
    )
    blob[:DE, _OFF_WOT : _OFF_WOT + DIN] = np.asarray(WO, np.float16).T
    blob[:DE, _OFF_B1] = np.asarray(b1, np.float16)
    blob[0, _OFF_BO : _OFF_BO + DIN] = np.asarray(bO, np.float16)
    return blob


def _tile_rows(xc):
    """[rows, 512] f16 -> [128, (rows/512)*2048] in (p, chunk, kt, s) order."""
    n = xc.shape[0] // 512
    return np.ascontiguousarray(
        xc.reshape(n, 512, 4, 128).transpose(3, 0, 2, 1)
    ).reshape(128, n * 2048)


def make_in_maps(x, W1, b1, W2, b2, W3, b3, WO, bO, rows=ROWS, n_cores=N_CORES):
    x = np.asarray(x, dtype=np.float32).astype(np.float16)
    total = x.shape[0] * x.shape[1]
    xf = x.reshape(total, DIN)
    blob = make_wconst(W1, b1, W2, b2, W3, b3, WO, bO)
    bvec = np.zeros((128, 2), np.float32)
    bvec[:, 0] = np.concatenate([np.asarray(b2, np.float32), np.asarray(b3, np.float32)])
    bvec[:DE, 1] = np.asarray(b1, np.float32)
    group = n_cores // 2
    batch_rows = rows * group
    in_maps = []
    for c in range(n_cores):
        b, q = divmod(c, group)
        xb = xf[b * batch_rows : (b + 1) * batch_rows]  # full batch of this core
        own = xb[q * rows : (q + 1) * rows]
        rest = np.concatenate([xb[: q * rows], xb[(q + 1) * rows :]], axis=0)
        m = {
            "wconst": blob,
            "bias32": bvec,
            "xt": np.concatenate([_tile_rows(own), _tile_rows(rest)], axis=1),
        }
        in_maps.append(m)
    return in_maps


def unshard_out(o, rows=ROWS):
    # o: [128, rows*4] f16 laid out (p, j, t, o) -> rows j*512 + t*128 + p
    n_chunks = rows // 512
    return (
        o.astype(np.float32)
        .reshape(128, n_chunks, 4, DIN)
        .transpose(1, 2, 0, 3)
        .reshape(rows, DIN)
    )


def kernel(x, W1, b1, W2, b2, W3, b3, WO, bO):
    global LAST_RESULT
    if "nc" not in _NC_CACHE:
        _NC_CACHE["nc"] = build_nc()
    nc = _NC_CACHE["nc"]
    in_maps = make_in_maps(x, W1, b1, W2, b2, W3, b3, WO, bO)
    res = run_bass_kernel_spmd(
        nc,
        in_maps,
        core_ids=list(range(N_CORES)),
        trace=TRACE,
        **TRACE_KWARGS,
    )
    LAST_RESULT = res
    full = np.concatenate(
        [unshard_out(res.results[c]["out"]) for c in range(N_CORES)], axis=0
    )  # [16384, 512] f32
    return full.reshape(BATCH, SEQ, DIN)
